# revision 51
# baseline (speedup 1.0000x reference)
"""Trainium2 Bass kernel for GQA attention (B=2, S=2048, HID=2048, H=16, G=4, D=128).

Sharding: 8 cores = 2 batches x 4 kv-groups. Core c handles batch c//4 and
kv-group c%4 (4 q heads + 1 kv head). Wq/Wk/Wv column-sharded by head group,
Wo row-sharded; per-core partial outputs are summed on the host per batch
(the unshard / all-reduce step).

v23-final (from v5 @258us -> ~238us): v5 compute structure, plus
- diag exp packing: key tile j of the diagonal slab covers q>=j*128, so
  (j=1, 384 cols) and (j=3, 128 cols) share one 512-col score tile and
  the diag costs 3 exp calls instead of 4 (ACT time quantizes per
  512-col unit); AV/den read the packed offsets, write true q ranges;
- chunk-1 q pairs allocate from the attention pool (attn(0) is tiny),
  decoupling them from chunk-0's rope chain in quad;
- x chunk0 piece 3 rides the scalar DMA queue (per-queue transfers
  serialize at ~3us per 512KB; 2MB on sync alone starved the k pass);
- last-chunk attention borrows the dead quad pool for score tiles
  (7-deep exp pipeline; no chunk-4 projections exist to conflict);
- hybrid fp8 attention: for off-diagonal key blocks the exp activation
  writes float8e4 directly (no vector casts) and A*V runs as fp8
  DoubleRow matmuls against an fp8 copy of V (2x PE rate); the diagonal
  block -- which carries the largest attention weights -- stays bf16, as
  does everything feeding the softmax (QK^T in bf16, fp32 accumulation);
- output projection of chunk c-1 interleaved per-slab into attention of
  chunk c between the score and AV matmuls: guaranteed-ready PE work that
  covers the scalar engine's exp latency in late chunks (PSUM->SBUF
  copies on vector, which lost its casts); last chunk's tiles drain at
  the tail with copies alternating scalar/vector, final two split;
- den PSUM tile allocated lazily at the head's denominator phase so the
  score tiles rotate through three PSUM slots instead of two;
- startup: wk then wq piece 0 head the scalar queue, x chunk0 alone on
  sync, chunk-0 runs k one x-piece ahead of q01 to absorb wq0 arrival;
  xpool bufs=6 aliases chunk c+2's x onto chunk c's slots, gating bulk
  prefetch (and wo, queued behind it) off the startup window.
Measured rel err 1.55e-2 (gate 2e-2); error is deterministic for the
fixed reference seed.
"""

import os
import sys

sys.path.insert(0, "/opt/trn_rl_repo")

import numpy as np

B, S, HID = 2, 2048, 2048
H, G, D = 16, 4, 128
REP = H // G  # q heads per kv head = 4
NEG = -1e30
SCALE = 1.0 / np.sqrt(np.float32(D))

NKT = HID // 128  # 16 contraction tiles for projections
NSC = S // 512  # 4 s-chunks of 512
NST = S // 128  # 16 s-tiles of 128


def _emit(nc, tc, aps):
    """Emit the per-core program. aps: dict of DRAM APs."""
    from contextlib import ExitStack

    from concourse import mybir

    f32 = mybir.dt.float32
    bf16 = mybir.dt.bfloat16
    fp8 = mybir.dt.float8e4
    DR = mybir.MatmulPerfMode.DoubleRow
    Exp = mybir.ActivationFunctionType.Exp

    # pre-tiled DRAM operands (partition dim first, contiguous free dims)
    xT = aps["xT"]  # [128, 4c, 4p, 4t, 512] bf16
    wq, wk, wv, wo = aps["wq"], aps["wk"], aps["wv"], aps["wo"]
    cosT, sinT = aps["cosT"], aps["sinT"]  # [128, 2048] bf16
    mask, ones, ones8 = aps["mask"], aps["ones"], aps["ones8"]
    out = aps["out"]

    lo = slice(0, 64)
    hi = slice(64, 128)

    with ExitStack() as ctx:
        persist = ctx.enter_context(tc.tile_pool(name="persist", bufs=1))

        # constants
        mask_t = persist.tile([128, 128], f32, tag="mask", name="mask_t")
        ones_t = persist.tile([128, 128], bf16, tag="ones", name="ones_t")
        ones8_t = persist.tile([128, 2, 128], fp8, tag="ones8", name="ones8_t")

        # weights
        wq_p = [
            persist.tile([128, 4, 512], bf16, tag=f"wq{i}", name=f"wq{i}")
            for i in range(4)
        ]
        wk_sb = persist.tile([128, 16, 128], bf16, tag="wk", name="wk_sb")
        wv_sb = persist.tile([128, 16, 128], bf16, tag="wv", name="wv_sb")
        wo_p = [
            persist.tile([128, 1, 2048], bf16, tag=f"wo{i}", name=f"wo{i}")
            for i in range(4)
        ]
        cos_p = [
            persist.tile([128, 1024], bf16, tag=f"cos{i}", name=f"cos{i}")
            for i in range(2)
        ]
        sin_p = [
            persist.tile([128, 1024], bf16, tag=f"sin{i}", name=f"sin{i}")
            for i in range(2)
        ]

        # persistent activations
        qt = [
            persist.tile([128, S], bf16, tag=f"qt{h}", name=f"qt{h}")
            for h in range(REP)
        ]
        kt = persist.tile([128, S], bf16, tag="kt", name="kt")
        vs = persist.tile([128, NST, 128], bf16, tag="vs", name="vs")
        vs8 = persist.tile([128, NST, 128], fp8, tag="vs8", name="vs8")
        aot = [
            persist.tile([128, S], bf16, tag=f"aot{h}", name=f"aot{h}")
            for h in range(REP)
        ]

        # xpool bufs=6: chunk c+1 pieces 2,3 and chunk c+2 alias chunk-c
        # slots, gating the bulk x prefetch off the startup critical path
        xpool = ctx.enter_context(tc.tile_pool(name="xsl", bufs=6))
        # quad: projection accumulators (independent of the attention pool so
        # chunk c+1 projections overlap attn(c)); pb: av/den/sp/ops
        quad = ctx.enter_context(tc.tile_pool(name="quad", bufs=4, space="PSUM"))
        pb = ctx.enter_context(tc.tile_pool(name="pb", bufs=4, space="PSUM"))
        epool = ctx.enter_context(tc.tile_pool(name="es", bufs=5))
        e8pool = ctx.enter_context(tc.tile_pool(name="es8", bufs=4))
        rpool = ctx.enter_context(tc.tile_pool(name="rope", bufs=4))
        rcp = ctx.enter_context(tc.tile_pool(name="rc", bufs=2))
        ocp = ctx.enter_context(tc.tile_pool(name="oc", bufs=4))

        # ---------------- PE p-state warm-up ----------------
        # the PE idles ~13us waiting for the first DMA bytes and then pays
        # a ~3us mid-p-state ramp; spin it on a locally-initialized scratch
        # tile so real work starts at full clock
        warm = persist.tile([128, 512], bf16, tag="warm", name="warm")
        nc.vector.memzero(warm[:])
        spin = pb.tile([128, 512], f32, tag="pb", name="spin")
        for _ in range(12):
            nc.tensor.matmul(
                spin[:], lhsT=warm[:, 0:128], rhs=warm[:], start=True, stop=True
            )

        # ---------------- startup DMA sequence ----------------
        # x slabs: per chunk, 4 piece-tiles of [128, 4, 512]
        xs = [
            [
                xpool.tile([128, 4, 512], bf16, tag="x", name=f"xs{c}_{p}")
                for p in range(4)
            ]
            for c in range(NSC)
        ]

        def dma_x_chunk(c, eng):
            for p in range(4):
                eng.dma_start(xs[c][p][:], xT[:, c, p, :, :])

        # scalar queue: wk first (gates the first matmul), then wq piece 0
        # (chunk-0 runs k one piece ahead of q01 to absorb wq0's arrival),
        # wv, trig, mask
        nc.scalar.dma_start(wk_sb[:], wk)
        nc.scalar.dma_start(wq_p[0][:], wq[:, 0:4, :])
        # x chunk0 piece 3 rides the scalar queue: 2MB serially on sync alone
        # (~3us per 512KB piece) starves the k pass
        nc.scalar.dma_start(xs[0][3][:], xT[:, 0, 3, :, :])
        nc.scalar.dma_start(wv_sb[:], wv)
        nc.scalar.dma_start(cos_p[0][:], cosT[:, 0:1024])
        nc.scalar.dma_start(sin_p[0][:], sinT[:, 0:1024])
        nc.scalar.dma_start(mask_t[:], mask)
        nc.scalar.dma_start(cos_p[1][:], cosT[:, 1024:2048])
        nc.scalar.dma_start(sin_p[1][:], sinT[:, 1024:2048])
        # sync queue: x chunk0 pieces 0-2 (piece 3 is on scalar)
        for p in range(3):
            nc.sync.dma_start(xs[0][p][:], xT[:, 0, p, :, :])
        # gpsimd queue: rest of wq (arrive piece-by-piece for the merged
        # pass), ones, x chunk1 (pieces 2,3 gated by xpool aliasing), then
        # wo behind them (in-order queue keeps wo out of the startup window)
        nc.gpsimd.dma_start(wq_p[1][:], wq[:, 4:8, :])
        nc.gpsimd.dma_start(wq_p[2][:], wq[:, 8:12, :])
        nc.gpsimd.dma_start(wq_p[3][:], wq[:, 12:16, :])
        nc.gpsimd.dma_start(ones_t[:], ones)
        nc.gpsimd.dma_start(ones8_t[:], ones8)
        dma_x_chunk(1, nc.gpsimd)
        for m in range(4):
            nc.gpsimd.dma_start(wo_p[m][:], wo[:, m : m + 1, :])

        def emit_optile(st_i, hc, copy_eng, op_pool=None):
            """One [128,512] output-projection tile: 4 accumulating matmuls,
            PSUM->SBUF copy on the chosen engine, DMA out."""
            ss = slice(st_i * 128, (st_i + 1) * 128)
            hs = slice(hc * 512, (hc + 1) * 512)
            if op_pool is None:
                ops = pb.tile([128, 512], f32, tag="pb", name="ops")
            else:
                ops = op_pool.tile([128, 512], f32, tag="qd", name="ops")
            for m in range(REP):
                nc.tensor.matmul(
                    ops[:],
                    lhsT=aot[m][:, ss],
                    rhs=wo_p[m][:, 0, hs],
                    start=(m == 0),
                    stop=(m == REP - 1),
                )
            oc = ocp.tile([128, 512], bf16, tag="oc", name="oc")
            if copy_eng == "s":
                nc.scalar.copy(oc[:], ops[:])
            elif copy_eng == "split":
                # final drain: halve latency by splitting the copy across
                # the scalar and vector engines
                nc.scalar.copy(oc[:, 0:256], ops[:, 0:256])
                nc.vector.tensor_copy(oc[:, 256:512], ops[:, 256:512])
            else:
                nc.vector.tensor_copy(oc[:], ops[:])
            nc.sync.dma_start(out[ss, hs], oc[:])

        def rope(acc, dest, cs_off, piece):
            """dest = acc*cos + swap_half(acc)*sin_signed  (sin pre-negated in
            its low half on the host)."""
            cp = cos_p[piece]
            sp_ = sin_p[piece]
            co = slice(cs_off, cs_off + 512)
            tmp_a = rpool.tile([128, 512], f32, tag="ra", name="tmp_a")
            tmp_b = rpool.tile([128, 512], f32, tag="rb", name="tmp_b")
            nc.vector.tensor_mul(tmp_b[lo, :], acc[hi, :], sp_[lo, co])
            nc.vector.tensor_mul(tmp_b[hi, :], acc[lo, :], sp_[hi, co])
            nc.vector.tensor_mul(tmp_a[:], acc[:], cp[:, co])
            nc.vector.tensor_add(dest, tmp_a[:], tmp_b[:])

        pend = []
        for c in range(NSC):
            cs = slice(c * 512, (c + 1) * 512)
            piece = c // 2
            cs_off = (c % 2) * 512

            # ---------------- projections ----------------
            if c == 0:
                # chunk 0 is paced by the x-piece DMAs: interleave k with
                # q0/q1 per piece (3 open accumulation groups) so the PE has
                # work per piece arrival
                k_acc = quad.tile([128, 512], f32, tag="qd", name="k_acc")
                q_a = quad.tile([128, 512], f32, tag="qd", name="q_acc00")
                q_b = quad.tile([128, 512], f32, tag="qd", name="q_acc01")
                q01 = [q_a, q_b]

                def q01_piece(p):
                    for tt in range(4):
                        t = 4 * p + tt
                        for j in range(2):
                            nc.tensor.matmul(
                                q01[j][:],
                                lhsT=wq_p[p][:, tt, j * 128 : (j + 1) * 128],
                                rhs=xs[0][p][:, tt, :],
                                start=(t == 0),
                                stop=(t == NKT - 1),
                            )

                # k runs one x-piece ahead of q01 so the PE has k work in
                # hand while wq piece 0 (second on the scalar queue) lands
                for p in range(4):
                    for tt in range(4):
                        t = 4 * p + tt
                        nc.tensor.matmul(
                            k_acc[:],
                            lhsT=wk_sb[:, t, :],
                            rhs=xs[0][p][:, tt, :],
                            start=(t == 0),
                            stop=(t == NKT - 1),
                        )
                    if p >= 1:
                        q01_piece(p - 1)
                q01_piece(3)
                rope(k_acc, kt[:, cs], cs_off, piece)
                vT_acc = quad.tile([128, 4, 128], f32, tag="qd", name="vT_acc")
                for i in range(4):
                    for t in range(NKT):
                        nc.tensor.matmul(
                            vT_acc[:, i, :],
                            lhsT=xs[0][t // 4][:, t % 4, i * 128 : (i + 1) * 128],
                            rhs=wv_sb[:, t, :],
                            start=(t == 0),
                            stop=(t == NKT - 1),
                        )
                rope(q01[0], qt[0][:, cs], cs_off, piece)
                rope(q01[1], qt[1][:, cs], cs_off, piece)
                nc.scalar.copy(vs[:, 0:4, :], vT_acc[:])
                nc.scalar.copy(vs8[:, 0:4, :], vT_acc[:])
                pairs = [1]
            else:
                # pass A: k and v^T (so RoPE-k and V land first for attn)
                k_acc = quad.tile([128, 512], f32, tag="qd", name="k_acc")
                for t in range(NKT):
                    nc.tensor.matmul(
                        k_acc[:],
                        lhsT=wk_sb[:, t, :],
                        rhs=xs[c][t // 4][:, t % 4, :],
                        start=(t == 0),
                        stop=(t == NKT - 1),
                    )
                vT_acc = quad.tile([128, 4, 128], f32, tag="qd", name="vT_acc")
                for i in range(4):
                    for t in range(NKT):
                        nc.tensor.matmul(
                            vT_acc[:, i, :],
                            lhsT=xs[c][t // 4][:, t % 4, i * 128 : (i + 1) * 128],
                            rhs=wv_sb[:, t, :],
                            start=(t == 0),
                            stop=(t == NKT - 1),
                        )
                rope(k_acc, kt[:, cs], cs_off, piece)
                nc.scalar.copy(vs[:, 4 * c : 4 * c + 4, :], vT_acc[:])
                nc.scalar.copy(vs8[:, 4 * c : 4 * c + 4, :], vT_acc[:])
                pairs = [0, 1]

            # remaining q pairs
            for pair in pairs:
                # chunk 1's pairs use the attention pool: attn(0) is tiny and
                # this avoids serializing on chunk-0's rope chain in quad
                qp_pool, qp_tag = (pb, "pb") if c == 1 else (quad, "qd")
                q_acc = [
                    qp_pool.tile(
                        [128, 512], f32, tag=qp_tag, name=f"q_acc{pair}{j}"
                    )
                    for j in range(2)
                ]
                for t in range(NKT):
                    for j in range(2):
                        h = 2 * pair + j
                        nc.tensor.matmul(
                            q_acc[j][:],
                            lhsT=wq_p[t // 4][:, t % 4, h * 128 : (h + 1) * 128],
                            rhs=xs[c][t // 4][:, t % 4, :],
                            start=(t == 0),
                            stop=(t == NKT - 1),
                        )
                for j in range(2):
                    h = 2 * pair + j
                    rope(q_acc[j], qt[h][:, cs], cs_off, piece)

            # prefetch x for chunk c+2 (gated by xpool slot aliasing)
            if c + 2 < NSC:
                dma_x_chunk(c + 2, nc.sync)

            # ---------------- attention for q-chunk c ----------------
            # outproj tiles of chunk c-1 are interleaved per-slab into this
            # chunk's attention: they are guaranteed-ready PE work that fills
            # the windows where the PE would wait on the scalar engine's exp
            # interleave the previous chunk's outproj tiles into this
            # chunk's attention as ready PE work covering the exp latency
            n_fill = len(pend)
            n_slots = REP * (c + 1)
            base, extra = divmod(n_fill, n_slots)
            # the spare tiles go to the diag slabs, whose exp chain leaves
            # the PE the most uncovered time
            n_op_slot = {}
            for h_ in range(REP):
                for kb_ in range(c + 1):
                    n_op_slot[(h_, kb_)] = base
            left = extra
            for kb_pref in [c] + list(range(c)):
                for h_ in range(REP):
                    if left > 0:
                        n_op_slot[(h_, kb_pref)] += 1
                        left -= 1
            for h in range(REP):
                av = pb.tile([128, 512], f32, tag="pb", name="av")
                es_diag = None
                es8_slabs = []
                # diag packing: key tile j covers q in [j*128, 512); pack
                # (j=1, 384 cols) with (j=3, 128 cols) into one 512-col sp
                # tile so the diag costs 3 exp calls instead of 4 (ACT time
                # is quantized per 512-col unit). A matmul's out AP and rhs
                # AP are independent, so AV/den read the packed offsets
                # while writing the true q ranges.
                DIAG_PACK = [  # (j, q0, width, sp tile idx, col offset)
                    (0, 0, 512, 0, 0),
                    (1, 128, 384, 1, 0),
                    (3, 384, 128, 1, 384),
                    (2, 256, 256, 2, 0),
                ]
                for kb in range(c + 1):
                    diag = kb == c
                    # in the last chunk no further projections exist, so the
                    # quad pool is dead: borrow it for score tiles to deepen
                    # the exp pipeline
                    sp_pool = quad if c == NSC - 1 else pb
                    sp_tag = "qd" if c == NSC - 1 else "pb"
                    if diag:
                        # diagonal slab stays bf16 (largest attention weights
                        # carry most of the precision budget)
                        es = epool.tile([128, 4, 512], bf16, tag="es", name="es")
                        es_diag = es
                        sp_tiles = []
                        for entry_i, (j, q0, w, ti, off) in enumerate(DIAG_PACK):
                            if ti == len(sp_tiles):
                                sp_tiles.append(
                                    sp_pool.tile(
                                        [128, 512], f32, tag=sp_tag, name="sp_t"
                                    )
                                )
                            sp_t = sp_tiles[ti]
                            i = kb * 4 + j
                            nc.tensor.matmul(
                                sp_t[:, off : off + w],
                                lhsT=kt[:, i * 128 : (i + 1) * 128],
                                rhs=qt[h][:, c * 512 + q0 : c * 512 + q0 + w],
                                start=True,
                                stop=True,
                            )
                            # each entry's valid q starts at its own diagonal
                            # block, so the mask lands on its first 128 cols
                            nc.vector.tensor_add(
                                sp_t[:, off : off + 128],
                                sp_t[:, off : off + 128],
                                mask_t[:],
                            )
                        for ti, cols in [(0, 512), (1, 512), (2, 256)]:
                            nc.scalar.activation(
                                es[:, ti, 0:cols],
                                sp_tiles[ti][:, 0:cols],
                                Exp,
                                scale=float(SCALE),
                            )
                    else:
                        # full slabs: exp writes fp8 directly; consumed by
                        # DoubleRow AV and denominator matmuls
                        es = e8pool.tile([128, 4, 512], fp8, tag="es8", name="es8")
                        es8_slabs.append(es)
                        for j in range(4):
                            i = kb * 4 + j
                            sp_t = sp_pool.tile(
                                [128, 512], f32, tag=sp_tag, name="sp_t"
                            )
                            nc.tensor.matmul(
                                sp_t[:],
                                lhsT=kt[:, i * 128 : (i + 1) * 128],
                                rhs=qt[h][:, c * 512 : (c + 1) * 512],
                                start=True,
                                stop=True,
                            )
                            nc.scalar.activation(
                                es[:, j, :],
                                sp_t[:],
                                Exp,
                                scale=float(SCALE),
                            )
                    # interleaved outproj tiles run between the scores and
                    # the AVs: guaranteed-ready PE work covering the exp
                    # latency (vector is light now, copies go there)
                    for _ in range(n_op_slot[(h, kb)]):
                        st, hc = pend.pop(0)
                        emit_optile(st, hc, copy_eng="v")
                    if diag:
                        for entry_i, (j, q0, w, ti, off) in enumerate(DIAG_PACK):
                            i = kb * 4 + j
                            nc.tensor.matmul(
                                av[:, q0 : q0 + w],
                                lhsT=vs[:, i, :],
                                rhs=es[:, ti, off : off + w],
                                start=(c == 0 and entry_i == 0),
                                stop=(entry_i == len(DIAG_PACK) - 1),
                            )
                    else:
                        for jj in range(2):
                            i = kb * 4 + 2 * jj
                            nc.tensor.matmul(
                                av[:, :],
                                lhsT=vs8[:, i : i + 2, :],
                                rhs=es[:, 2 * jj : 2 * jj + 2, :],
                                start=(kb == 0 and jj == 0),
                                stop=False,
                                perf_mode=DR,
                            )
                # denominator phase: DoubleRow fp8 for full groups, bf16 for
                # the diagonal; kept off the sc/av critical path. den is
                # allocated here (not at head start) so the scores/AV phase
                # has three sp slots to rotate through instead of two
                den = pb.tile([128, 512], f32, tag="pb", name="den")
                for kb in range(c):
                    for pair in range(2):
                        nc.tensor.matmul(
                            den[:, :],
                            lhsT=ones8_t[:],
                            rhs=es8_slabs[kb][:, 2 * pair : 2 * pair + 2, :],
                            start=(kb == 0 and pair == 0),
                            stop=False,
                            perf_mode=DR,
                        )
                for entry_i, (j, q0, w, ti, off) in enumerate(DIAG_PACK):
                    nc.tensor.matmul(
                        den[:, q0 : q0 + w],
                        lhsT=ones_t[:],
                        rhs=es_diag[:, ti, off : off + w],
                        start=(c == 0 and entry_i == 0),
                        stop=(entry_i == len(DIAG_PACK) - 1),
                    )
                rc = rcp.tile([128, 512], f32, tag="rc", name="rc")
                nc.vector.reciprocal_approx_fast(rc[:], den[:])
                nc.vector.tensor_mul(aot[h][:, cs], av[:], rc[:])

            # this chunk's outproj tiles become ready now
            pend.extend(
                (st, hc) for st in range(4 * c, 4 * c + 4) for hc in range(NSC)
            )

            # ---------------- output projection tail (last chunk: whatever
            # the interleave didn't drain) ---------------------------------
            if c == NSC - 1:
                n_tail = len(pend)
                for i_t in range(n_tail):
                    st_i, hc = pend.pop(0)
                    last_two = i_t >= n_tail - 2
                    if last_two:
                        eng = "split"
                    elif (st_i + hc) % 2 == 0:
                        eng = "s"
                    else:
                        eng = "v"
                    # quad is free in the tail too: alternate pools for a
                    # deeper ops rotation during the final drain
                    emit_optile(
                        st_i, hc, copy_eng=eng,
                        op_pool=quad if i_t % 2 else None,
                    )


def build_program():
    import concourse.tile as tile
    from concourse import bacc, mybir

    f32 = mybir.dt.float32
    bf16 = mybir.dt.bfloat16
    nc = bacc.Bacc("TRN2", target_bir_lowering=False, debug=False, num_devices=8)
    aps = {}
    aps["xT"] = nc.dram_tensor(
        "xT", [128, 4, 4, 4, 512], bf16, kind="ExternalInput"
    ).ap()
    aps["cosT"] = nc.dram_tensor("cosT", [D, S], bf16, kind="ExternalInput").ap()
    aps["sinT"] = nc.dram_tensor("sinT", [D, S], bf16, kind="ExternalInput").ap()
    aps["wq"] = nc.dram_tensor("wq", [128, 16, 512], bf16, kind="ExternalInput").ap()
    aps["wk"] = nc.dram_tensor("wk", [128, 16, 128], bf16, kind="ExternalInput").ap()
    aps["wv"] = nc.dram_tensor("wv", [128, 16, 128], bf16, kind="ExternalInput").ap()
    aps["wo"] = nc.dram_tensor("wo", [128, 4, 2048], bf16, kind="ExternalInput").ap()
    aps["mask"] = nc.dram_tensor("mask", [128, 128], f32, kind="ExternalInput").ap()
    aps["ones"] = nc.dram_tensor("ones", [128, 128], bf16, kind="ExternalInput").ap()
    aps["ones8"] = nc.dram_tensor(
        "ones8", [128, 256], mybir.dt.float8e4, kind="ExternalInput"
    ).ap()
    aps["out"] = nc.dram_tensor("out", [S, HID], bf16, kind="ExternalOutput").ap()

    with tile.TileContext(nc) as tc:
        _emit(nc, tc, aps)
    nc.compile()
    return nc


def _tile_kdim(w):
    """[K, M] -> [128, K//128, M] with element (p, t, m) = w[t*128+p, m]."""
    K, M = w.shape
    return np.ascontiguousarray(w.reshape(K // 128, 128, M).transpose(1, 0, 2))


def make_in_maps(x, cos, sin, Wq, Wk, Wv, Wo):
    """Build the 8 per-core input dicts. Core c: batch c//4, kv-group c%4."""
    import ml_dtypes

    bf = ml_dtypes.bfloat16
    mask = np.where(
        np.arange(128)[:, None] <= np.arange(128)[None, :], 0.0, NEG
    ).astype(np.float32)
    ones = np.ones((128, 128), dtype=bf)
    ones8 = np.ones((128, 256), dtype=ml_dtypes.float8_e4m3)
    # x^T pre-tiled: [128, chunk(4), piece(4), t_in_piece(4), s_in_chunk(512)]
    # with hid = (piece*4 + t)*128 + p and s = chunk*512 + s'.
    xT = []
    for b in range(B):
        A = np.ascontiguousarray(x[b].T).astype(bf)  # [2048 hid, 2048 s]
        A = A.reshape(4, 4, 128, 4, 512)  # [piece, t, p, chunk, s']
        xT.append(np.ascontiguousarray(A.transpose(2, 3, 0, 1, 4)))
    cosT = np.ascontiguousarray(cos.T).astype(bf)
    sinT = np.ascontiguousarray(sin.T).astype(np.float32)
    sinT[0:64, :] *= -1.0  # sign-fold rotate_half's negation into the table
    sinT = sinT.astype(bf)
    in_maps = []
    for c in range(8):
        b, g = c // 4, c % 4
        in_maps.append(
            {
                "xT": xT[b],
                "cosT": cosT,
                "sinT": sinT,
                "wq": _tile_kdim(Wq[:, g * REP * D : (g + 1) * REP * D]).astype(bf),
                "wk": _tile_kdim(Wk[:, g * D : (g + 1) * D]).astype(bf),
                "wv": _tile_kdim(Wv[:, g * D : (g + 1) * D]).astype(bf),
                "wo": _tile_kdim(Wo[g * REP * D : (g + 1) * REP * D, :]).astype(bf),
                "mask": mask,
                "ones": ones,
                "ones8": ones8,
            }
        )
    return in_maps


def kernel(x, cos, sin, Wq, Wk, Wv, Wo):
    from concourse import bass_utils

    nc = build_program()
    in_maps = make_in_maps(x, cos, sin, Wq, Wk, Wv, Wo)
    trace = bool(int(os.environ.get("BASS_KERNEL_TRACE", "0")))
    res = bass_utils.run_bass_kernel_spmd(
        nc,
        in_maps,
        core_ids=list(range(8)),
        trace=trace,
    )
    if trace:
        print(f"HW exec time: {res.exec_time_ns} ns")
        if res.instructions_and_trace is not None:
            print(f"trace: {res.instructions_and_trace[1]}")
    out = np.empty((B, S, HID), dtype=np.float32)
    for b in range(B):
        acc = res.results[4 * b]["out"].astype(np.float32)
        for g in range(1, G):
            acc = acc + res.results[4 * b + g]["out"].astype(np.float32)
        out[b] = acc
    return out


# revision 52
# speedup vs baseline: 1.0170x; 1.0170x over previous
"""Trainium2 Bass kernel for GQA attention (B=2, S=2048, HID=2048, H=16, G=4, D=128).

Sharding: 8 cores = 2 batches x 4 kv-groups. Core c handles batch c//4 and
kv-group c%4 (4 q heads + 1 kv head). Wq/Wk/Wv column-sharded by head group,
Wo row-sharded; per-core partial outputs are summed on the host per batch
(the unshard / all-reduce step).

v17-final (from v5 @258us -> ~238us): v5 compute structure, plus
- x chunk0 piece 3 rides the scalar DMA queue (per-queue transfers
  serialize at ~3us per 512KB; 2MB on sync alone starved the k pass);
- last-chunk attention borrows the dead quad pool for score tiles
  (7-deep exp pipeline; no chunk-4 projections exist to conflict);
- hybrid fp8 attention: for off-diagonal key blocks the exp activation
  writes float8e4 directly (no vector casts) and A*V runs as fp8
  DoubleRow matmuls against an fp8 copy of V (2x PE rate); the diagonal
  block -- which carries the largest attention weights -- stays bf16, as
  does everything feeding the softmax (QK^T in bf16, fp32 accumulation);
- output projection of chunk c-1 interleaved per-slab into attention of
  chunk c between the score and AV matmuls: guaranteed-ready PE work that
  covers the scalar engine's exp latency in late chunks (PSUM->SBUF
  copies on vector, which lost its casts); last chunk's tiles drain at
  the tail with copies alternating scalar/vector, final two split;
- den PSUM tile allocated lazily at the head's denominator phase so the
  score tiles rotate through three PSUM slots instead of two;
- startup: wk then wq piece 0 head the scalar queue, x chunk0 alone on
  sync, chunk-0 runs k one x-piece ahead of q01 to absorb wq0 arrival;
  xpool bufs=6 aliases chunk c+2's x onto chunk c's slots, gating bulk
  prefetch (and wo, queued behind it) off the startup window.
Measured rel err 1.55e-2 (gate 2e-2); error is deterministic for the
fixed reference seed.
"""

import os
import sys

sys.path.insert(0, "/opt/trn_rl_repo")

import numpy as np

B, S, HID = 2, 2048, 2048
H, G, D = 16, 4, 128
REP = H // G  # q heads per kv head = 4
NEG = -1e30
SCALE = 1.0 / np.sqrt(np.float32(D))

NKT = HID // 128  # 16 contraction tiles for projections
NSC = S // 512  # 4 s-chunks of 512
NST = S // 128  # 16 s-tiles of 128


def _emit(nc, tc, aps):
    """Emit the per-core program. aps: dict of DRAM APs."""
    from contextlib import ExitStack

    from concourse import mybir

    f32 = mybir.dt.float32
    bf16 = mybir.dt.bfloat16
    fp8 = mybir.dt.float8e4
    DR = mybir.MatmulPerfMode.DoubleRow
    Exp = mybir.ActivationFunctionType.Exp

    # pre-tiled DRAM operands (partition dim first, contiguous free dims)
    xT = aps["xT"]  # [128, 4c, 4p, 4t, 512] bf16
    wq, wk, wv, wo = aps["wq"], aps["wk"], aps["wv"], aps["wo"]
    cosT, sinT = aps["cosT"], aps["sinT"]  # [128, 2048] bf16
    mask, ones, ones8 = aps["mask"], aps["ones"], aps["ones8"]
    out = aps["out"]

    lo = slice(0, 64)
    hi = slice(64, 128)

    with ExitStack() as ctx:
        persist = ctx.enter_context(tc.tile_pool(name="persist", bufs=1))

        # constants
        mask_t = persist.tile([128, 128], f32, tag="mask", name="mask_t")
        ones_t = persist.tile([128, 128], bf16, tag="ones", name="ones_t")
        ones8_t = persist.tile([128, 2, 128], fp8, tag="ones8", name="ones8_t")

        # weights
        wq_p = [
            persist.tile([128, 4, 512], bf16, tag=f"wq{i}", name=f"wq{i}")
            for i in range(4)
        ]
        wk_sb = persist.tile([128, 16, 128], bf16, tag="wk", name="wk_sb")
        wv_sb = persist.tile([128, 16, 128], bf16, tag="wv", name="wv_sb")
        wo_p = [
            persist.tile([128, 1, 2048], bf16, tag=f"wo{i}", name=f"wo{i}")
            for i in range(4)
        ]
        cos_p = [
            persist.tile([128, 1024], bf16, tag=f"cos{i}", name=f"cos{i}")
            for i in range(2)
        ]
        sin_p = [
            persist.tile([128, 1024], bf16, tag=f"sin{i}", name=f"sin{i}")
            for i in range(2)
        ]

        # persistent activations
        qt = [
            persist.tile([128, S], bf16, tag=f"qt{h}", name=f"qt{h}")
            for h in range(REP)
        ]
        kt = persist.tile([128, S], bf16, tag="kt", name="kt")
        vs = persist.tile([128, NST, 128], bf16, tag="vs", name="vs")
        vs8 = persist.tile([128, NST, 128], fp8, tag="vs8", name="vs8")
        aot = [
            persist.tile([128, S], bf16, tag=f"aot{h}", name=f"aot{h}")
            for h in range(REP)
        ]

        # xpool bufs=6: chunk c+1 pieces 2,3 and chunk c+2 alias chunk-c
        # slots, gating the bulk x prefetch off the startup critical path
        xpool = ctx.enter_context(tc.tile_pool(name="xsl", bufs=6))
        # quad: projection accumulators (independent of the attention pool so
        # chunk c+1 projections overlap attn(c)); pb: av/den/sp/ops
        quad = ctx.enter_context(tc.tile_pool(name="quad", bufs=4, space="PSUM"))
        pb = ctx.enter_context(tc.tile_pool(name="pb", bufs=4, space="PSUM"))
        epool = ctx.enter_context(tc.tile_pool(name="es", bufs=5))
        e8pool = ctx.enter_context(tc.tile_pool(name="es8", bufs=4))
        rpool = ctx.enter_context(tc.tile_pool(name="rope", bufs=4))
        rcp = ctx.enter_context(tc.tile_pool(name="rc", bufs=2))
        ocp = ctx.enter_context(tc.tile_pool(name="oc", bufs=4))

        # ---------------- startup DMA sequence ----------------
        # x slabs: per chunk, 4 piece-tiles of [128, 4, 512]
        xs = [
            [
                xpool.tile([128, 4, 512], bf16, tag="x", name=f"xs{c}_{p}")
                for p in range(4)
            ]
            for c in range(NSC)
        ]

        def dma_x_chunk(c, eng):
            for p in range(4):
                eng.dma_start(xs[c][p][:], xT[:, c, p, :, :])

        # scalar queue: wk first (gates the first matmul), then wq piece 0
        # (chunk-0 runs k one piece ahead of q01 to absorb wq0's arrival),
        # wv, trig, mask
        nc.scalar.dma_start(wk_sb[:], wk)
        nc.scalar.dma_start(wq_p[0][:], wq[:, 0:4, :])
        # x chunk0 piece 3 rides the scalar queue: 2MB serially on sync alone
        # (~3us per 512KB piece) starves the k pass
        nc.scalar.dma_start(xs[0][3][:], xT[:, 0, 3, :, :])
        nc.scalar.dma_start(wv_sb[:], wv)
        nc.scalar.dma_start(cos_p[0][:], cosT[:, 0:1024])
        nc.scalar.dma_start(sin_p[0][:], sinT[:, 0:1024])
        nc.scalar.dma_start(mask_t[:], mask)
        nc.scalar.dma_start(cos_p[1][:], cosT[:, 1024:2048])
        nc.scalar.dma_start(sin_p[1][:], sinT[:, 1024:2048])
        # sync queue: x chunk0 pieces 0-2 (piece 3 is on scalar)
        for p in range(3):
            nc.sync.dma_start(xs[0][p][:], xT[:, 0, p, :, :])
        # gpsimd queue: rest of wq (arrive piece-by-piece for the merged
        # pass), ones, x chunk1 (pieces 2,3 gated by xpool aliasing), then
        # wo behind them (in-order queue keeps wo out of the startup window)
        nc.gpsimd.dma_start(wq_p[1][:], wq[:, 4:8, :])
        nc.gpsimd.dma_start(wq_p[2][:], wq[:, 8:12, :])
        nc.gpsimd.dma_start(wq_p[3][:], wq[:, 12:16, :])
        nc.gpsimd.dma_start(ones_t[:], ones)
        nc.gpsimd.dma_start(ones8_t[:], ones8)
        dma_x_chunk(1, nc.gpsimd)
        for m in range(4):
            nc.gpsimd.dma_start(wo_p[m][:], wo[:, m : m + 1, :])

        def emit_optile(st_i, hc, copy_eng):
            """One [128,512] output-projection tile: 4 accumulating matmuls,
            PSUM->SBUF copy on the chosen engine, DMA out."""
            ss = slice(st_i * 128, (st_i + 1) * 128)
            hs = slice(hc * 512, (hc + 1) * 512)
            ops = pb.tile([128, 512], f32, tag="pb", name="ops")
            for m in range(REP):
                nc.tensor.matmul(
                    ops[:],
                    lhsT=aot[m][:, ss],
                    rhs=wo_p[m][:, 0, hs],
                    start=(m == 0),
                    stop=(m == REP - 1),
                )
            oc = ocp.tile([128, 512], bf16, tag="oc", name="oc")
            if copy_eng == "s":
                nc.scalar.copy(oc[:], ops[:])
            elif copy_eng == "split":
                # final drain: halve latency by splitting the copy across
                # the scalar and vector engines
                nc.scalar.copy(oc[:, 0:256], ops[:, 0:256])
                nc.vector.tensor_copy(oc[:, 256:512], ops[:, 256:512])
            else:
                nc.vector.tensor_copy(oc[:], ops[:])
            nc.sync.dma_start(out[ss, hs], oc[:])

        def rope(acc, dest, cs_off, piece):
            """dest = acc*cos + swap_half(acc)*sin_signed  (sin pre-negated in
            its low half on the host)."""
            cp = cos_p[piece]
            sp_ = sin_p[piece]
            co = slice(cs_off, cs_off + 512)
            tmp_a = rpool.tile([128, 512], f32, tag="ra", name="tmp_a")
            tmp_b = rpool.tile([128, 512], f32, tag="rb", name="tmp_b")
            nc.vector.tensor_mul(tmp_b[lo, :], acc[hi, :], sp_[lo, co])
            nc.vector.tensor_mul(tmp_b[hi, :], acc[lo, :], sp_[hi, co])
            nc.vector.tensor_mul(tmp_a[:], acc[:], cp[:, co])
            nc.vector.tensor_add(dest, tmp_a[:], tmp_b[:])

        pend = []
        for c in range(NSC):
            cs = slice(c * 512, (c + 1) * 512)
            piece = c // 2
            cs_off = (c % 2) * 512

            # ---------------- projections ----------------
            if c == 0:
                # chunk 0 is paced by the x-piece DMAs: interleave k with
                # q0/q1 per piece (3 open accumulation groups) so the PE has
                # work per piece arrival
                k_acc = quad.tile([128, 512], f32, tag="qd", name="k_acc")
                q_a = quad.tile([128, 512], f32, tag="qd", name="q_acc00")
                q_b = quad.tile([128, 512], f32, tag="qd", name="q_acc01")
                q01 = [q_a, q_b]

                def q01_piece(p):
                    for tt in range(4):
                        t = 4 * p + tt
                        for j in range(2):
                            nc.tensor.matmul(
                                q01[j][:],
                                lhsT=wq_p[p][:, tt, j * 128 : (j + 1) * 128],
                                rhs=xs[0][p][:, tt, :],
                                start=(t == 0),
                                stop=(t == NKT - 1),
                            )

                # k runs one x-piece ahead of q01 so the PE has k work in
                # hand while wq piece 0 (second on the scalar queue) lands
                for p in range(4):
                    for tt in range(4):
                        t = 4 * p + tt
                        nc.tensor.matmul(
                            k_acc[:],
                            lhsT=wk_sb[:, t, :],
                            rhs=xs[0][p][:, tt, :],
                            start=(t == 0),
                            stop=(t == NKT - 1),
                        )
                    if p >= 1:
                        q01_piece(p - 1)
                q01_piece(3)
                rope(k_acc, kt[:, cs], cs_off, piece)
                vT_acc = quad.tile([128, 4, 128], f32, tag="qd", name="vT_acc")
                for i in range(4):
                    for t in range(NKT):
                        nc.tensor.matmul(
                            vT_acc[:, i, :],
                            lhsT=xs[0][t // 4][:, t % 4, i * 128 : (i + 1) * 128],
                            rhs=wv_sb[:, t, :],
                            start=(t == 0),
                            stop=(t == NKT - 1),
                        )
                rope(q01[0], qt[0][:, cs], cs_off, piece)
                rope(q01[1], qt[1][:, cs], cs_off, piece)
                nc.scalar.copy(vs[:, 0:4, :], vT_acc[:])
                nc.scalar.copy(vs8[:, 0:4, :], vT_acc[:])
                pairs = [1]
            else:
                # pass A: k and v^T (so RoPE-k and V land first for attn)
                k_acc = quad.tile([128, 512], f32, tag="qd", name="k_acc")
                for t in range(NKT):
                    nc.tensor.matmul(
                        k_acc[:],
                        lhsT=wk_sb[:, t, :],
                        rhs=xs[c][t // 4][:, t % 4, :],
                        start=(t == 0),
                        stop=(t == NKT - 1),
                    )
                vT_acc = quad.tile([128, 4, 128], f32, tag="qd", name="vT_acc")
                for i in range(4):
                    for t in range(NKT):
                        nc.tensor.matmul(
                            vT_acc[:, i, :],
                            lhsT=xs[c][t // 4][:, t % 4, i * 128 : (i + 1) * 128],
                            rhs=wv_sb[:, t, :],
                            start=(t == 0),
                            stop=(t == NKT - 1),
                        )
                rope(k_acc, kt[:, cs], cs_off, piece)
                nc.scalar.copy(vs[:, 4 * c : 4 * c + 4, :], vT_acc[:])
                nc.scalar.copy(vs8[:, 4 * c : 4 * c + 4, :], vT_acc[:])
                pairs = [0, 1]

            # remaining q pairs
            for pair in pairs:
                # chunk 1's pairs use the attention pool: attn(0) is tiny and
                # this avoids serializing on chunk-0's rope chain in quad
                qp_pool, qp_tag = (pb, "pb") if c == 1 else (quad, "qd")
                q_acc = [
                    qp_pool.tile(
                        [128, 512], f32, tag=qp_tag, name=f"q_acc{pair}{j}"
                    )
                    for j in range(2)
                ]
                for t in range(NKT):
                    for j in range(2):
                        h = 2 * pair + j
                        nc.tensor.matmul(
                            q_acc[j][:],
                            lhsT=wq_p[t // 4][:, t % 4, h * 128 : (h + 1) * 128],
                            rhs=xs[c][t // 4][:, t % 4, :],
                            start=(t == 0),
                            stop=(t == NKT - 1),
                        )
                for j in range(2):
                    h = 2 * pair + j
                    rope(q_acc[j], qt[h][:, cs], cs_off, piece)

            # prefetch x for chunk c+2 (gated by xpool slot aliasing)
            if c + 2 < NSC:
                dma_x_chunk(c + 2, nc.sync)

            # ---------------- attention for q-chunk c ----------------
            # outproj tiles of chunk c-1 are interleaved per-slab into this
            # chunk's attention: they are guaranteed-ready PE work that fills
            # the windows where the PE would wait on the scalar engine's exp
            # interleave the previous chunk's outproj tiles into this
            # chunk's attention as ready PE work covering the exp latency
            n_fill = len(pend)
            n_slots = REP * (c + 1)
            base, extra = divmod(n_fill, n_slots)
            # the spare tiles go to the diag slabs, whose exp chain leaves
            # the PE the most uncovered time
            n_op_slot = {}
            for h_ in range(REP):
                for kb_ in range(c + 1):
                    n_op_slot[(h_, kb_)] = base
            left = extra
            for kb_pref in [c] + list(range(c)):
                for h_ in range(REP):
                    if left > 0:
                        n_op_slot[(h_, kb_pref)] += 1
                        left -= 1
            for h in range(REP):
                av = pb.tile([128, 512], f32, tag="pb", name="av")
                es_diag = None
                es8_slabs = []
                # diag packing: key tile j covers q in [j*128, 512); pack
                # (j=1, 384 cols) with (j=3, 128 cols) into one 512-col sp
                # tile so the diag costs 3 exp calls instead of 4 (ACT time
                # is quantized per 512-col unit). A matmul's out AP and rhs
                # AP are independent, so AV/den read the packed offsets
                # while writing the true q ranges.
                DIAG_PACK = [  # (j, q0, width, sp tile idx, col offset)
                    (0, 0, 512, 0, 0),
                    (1, 128, 384, 1, 0),
                    (3, 384, 128, 1, 384),
                    (2, 256, 256, 2, 0),
                ]
                for kb in range(c + 1):
                    diag = kb == c
                    # in the last chunk no further projections exist, so the
                    # quad pool is dead: borrow it for score tiles to deepen
                    # the exp pipeline
                    sp_pool = quad if c == NSC - 1 else pb
                    sp_tag = "qd" if c == NSC - 1 else "pb"
                    if diag:
                        # diagonal slab stays bf16 (largest attention weights
                        # carry most of the precision budget)
                        es = epool.tile([128, 4, 512], bf16, tag="es", name="es")
                        es_diag = es
                        sp_tiles = []
                        for entry_i, (j, q0, w, ti, off) in enumerate(DIAG_PACK):
                            if ti == len(sp_tiles):
                                sp_tiles.append(
                                    sp_pool.tile(
                                        [128, 512], f32, tag=sp_tag, name="sp_t"
                                    )
                                )
                            sp_t = sp_tiles[ti]
                            i = kb * 4 + j
                            nc.tensor.matmul(
                                sp_t[:, off : off + w],
                                lhsT=kt[:, i * 128 : (i + 1) * 128],
                                rhs=qt[h][:, c * 512 + q0 : c * 512 + q0 + w],
                                start=True,
                                stop=True,
                            )
                            # each entry's valid q starts at its own diagonal
                            # block, so the mask lands on its first 128 cols
                            nc.vector.tensor_add(
                                sp_t[:, off : off + 128],
                                sp_t[:, off : off + 128],
                                mask_t[:],
                            )
                        for ti, cols in [(0, 512), (1, 512), (2, 256)]:
                            nc.scalar.activation(
                                es[:, ti, 0:cols],
                                sp_tiles[ti][:, 0:cols],
                                Exp,
                                scale=float(SCALE),
                            )
                    else:
                        # full slabs: exp writes fp8 directly; consumed by
                        # DoubleRow AV and denominator matmuls
                        es = e8pool.tile([128, 4, 512], fp8, tag="es8", name="es8")
                        es8_slabs.append(es)
                        for j in range(4):
                            i = kb * 4 + j
                            sp_t = sp_pool.tile(
                                [128, 512], f32, tag=sp_tag, name="sp_t"
                            )
                            nc.tensor.matmul(
                                sp_t[:],
                                lhsT=kt[:, i * 128 : (i + 1) * 128],
                                rhs=qt[h][:, c * 512 : (c + 1) * 512],
                                start=True,
                                stop=True,
                            )
                            nc.scalar.activation(
                                es[:, j, :],
                                sp_t[:],
                                Exp,
                                scale=float(SCALE),
                            )
                    # interleaved outproj tiles run between the scores and
                    # the AVs: guaranteed-ready PE work covering the exp
                    # latency (vector is light now, copies go there)
                    for _ in range(n_op_slot[(h, kb)]):
                        st, hc = pend.pop(0)
                        emit_optile(st, hc, copy_eng="v")
                    if diag:
                        for entry_i, (j, q0, w, ti, off) in enumerate(DIAG_PACK):
                            i = kb * 4 + j
                            nc.tensor.matmul(
                                av[:, q0 : q0 + w],
                                lhsT=vs[:, i, :],
                                rhs=es[:, ti, off : off + w],
                                start=(c == 0 and entry_i == 0),
                                stop=(entry_i == len(DIAG_PACK) - 1),
                            )
                    else:
                        for jj in range(2):
                            i = kb * 4 + 2 * jj
                            nc.tensor.matmul(
                                av[:, :],
                                lhsT=vs8[:, i : i + 2, :],
                                rhs=es[:, 2 * jj : 2 * jj + 2, :],
                                start=(kb == 0 and jj == 0),
                                stop=False,
                                perf_mode=DR,
                            )
                # denominator phase: DoubleRow fp8 for full groups, bf16 for
                # the diagonal; kept off the sc/av critical path. den is
                # allocated here (not at head start) so the scores/AV phase
                # has three sp slots to rotate through instead of two
                den = pb.tile([128, 512], f32, tag="pb", name="den")
                for kb in range(c):
                    for pair in range(2):
                        nc.tensor.matmul(
                            den[:, :],
                            lhsT=ones8_t[:],
                            rhs=es8_slabs[kb][:, 2 * pair : 2 * pair + 2, :],
                            start=(kb == 0 and pair == 0),
                            stop=False,
                            perf_mode=DR,
                        )
                for entry_i, (j, q0, w, ti, off) in enumerate(DIAG_PACK):
                    nc.tensor.matmul(
                        den[:, q0 : q0 + w],
                        lhsT=ones_t[:],
                        rhs=es_diag[:, ti, off : off + w],
                        start=(c == 0 and entry_i == 0),
                        stop=(entry_i == len(DIAG_PACK) - 1),
                    )
                rc = rcp.tile([128, 512], f32, tag="rc", name="rc")
                nc.vector.reciprocal_approx_fast(rc[:], den[:])
                nc.vector.tensor_mul(aot[h][:, cs], av[:], rc[:])

            # this chunk's outproj tiles become ready now
            pend.extend(
                (st, hc) for st in range(4 * c, 4 * c + 4) for hc in range(NSC)
            )

            # ---------------- output projection tail (last chunk: whatever
            # the interleave didn't drain) ---------------------------------
            if c == NSC - 1:
                n_tail = len(pend)
                for i_t in range(n_tail):
                    st_i, hc = pend.pop(0)
                    last_two = i_t >= n_tail - 2
                    if last_two:
                        eng = "split"
                    elif (st_i + hc) % 2 == 0:
                        eng = "s"
                    else:
                        eng = "v"
                    emit_optile(st_i, hc, copy_eng=eng)


def build_program():
    import concourse.tile as tile
    from concourse import bacc, mybir

    f32 = mybir.dt.float32
    bf16 = mybir.dt.bfloat16
    nc = bacc.Bacc("TRN2", target_bir_lowering=False, debug=False, num_devices=8)
    aps = {}
    aps["xT"] = nc.dram_tensor(
        "xT", [128, 4, 4, 4, 512], bf16, kind="ExternalInput"
    ).ap()
    aps["cosT"] = nc.dram_tensor("cosT", [D, S], bf16, kind="ExternalInput").ap()
    aps["sinT"] = nc.dram_tensor("sinT", [D, S], bf16, kind="ExternalInput").ap()
    aps["wq"] = nc.dram_tensor("wq", [128, 16, 512], bf16, kind="ExternalInput").ap()
    aps["wk"] = nc.dram_tensor("wk", [128, 16, 128], bf16, kind="ExternalInput").ap()
    aps["wv"] = nc.dram_tensor("wv", [128, 16, 128], bf16, kind="ExternalInput").ap()
    aps["wo"] = nc.dram_tensor("wo", [128, 4, 2048], bf16, kind="ExternalInput").ap()
    aps["mask"] = nc.dram_tensor("mask", [128, 128], f32, kind="ExternalInput").ap()
    aps["ones"] = nc.dram_tensor("ones", [128, 128], bf16, kind="ExternalInput").ap()
    aps["ones8"] = nc.dram_tensor(
        "ones8", [128, 256], mybir.dt.float8e4, kind="ExternalInput"
    ).ap()
    aps["out"] = nc.dram_tensor("out", [S, HID], bf16, kind="ExternalOutput").ap()

    with tile.TileContext(nc) as tc:
        _emit(nc, tc, aps)
    nc.compile()
    return nc


def _tile_kdim(w):
    """[K, M] -> [128, K//128, M] with element (p, t, m) = w[t*128+p, m]."""
    K, M = w.shape
    return np.ascontiguousarray(w.reshape(K // 128, 128, M).transpose(1, 0, 2))


def make_in_maps(x, cos, sin, Wq, Wk, Wv, Wo):
    """Build the 8 per-core input dicts. Core c: batch c//4, kv-group c%4."""
    import ml_dtypes

    bf = ml_dtypes.bfloat16
    mask = np.where(
        np.arange(128)[:, None] <= np.arange(128)[None, :], 0.0, NEG
    ).astype(np.float32)
    ones = np.ones((128, 128), dtype=bf)
    ones8 = np.ones((128, 256), dtype=ml_dtypes.float8_e4m3)
    # x^T pre-tiled: [128, chunk(4), piece(4), t_in_piece(4), s_in_chunk(512)]
    # with hid = (piece*4 + t)*128 + p and s = chunk*512 + s'.
    xT = []
    for b in range(B):
        A = np.ascontiguousarray(x[b].T).astype(bf)  # [2048 hid, 2048 s]
        A = A.reshape(4, 4, 128, 4, 512)  # [piece, t, p, chunk, s']
        xT.append(np.ascontiguousarray(A.transpose(2, 3, 0, 1, 4)))
    cosT = np.ascontiguousarray(cos.T).astype(bf)
    sinT = np.ascontiguousarray(sin.T).astype(np.float32)
    sinT[0:64, :] *= -1.0  # sign-fold rotate_half's negation into the table
    sinT = sinT.astype(bf)
    in_maps = []
    for c in range(8):
        b, g = c // 4, c % 4
        in_maps.append(
            {
                "xT": xT[b],
                "cosT": cosT,
                "sinT": sinT,
                "wq": _tile_kdim(Wq[:, g * REP * D : (g + 1) * REP * D]).astype(bf),
                "wk": _tile_kdim(Wk[:, g * D : (g + 1) * D]).astype(bf),
                "wv": _tile_kdim(Wv[:, g * D : (g + 1) * D]).astype(bf),
                "wo": _tile_kdim(Wo[g * REP * D : (g + 1) * REP * D, :]).astype(bf),
                "mask": mask,
                "ones": ones,
                "ones8": ones8,
            }
        )
    return in_maps


def kernel(x, cos, sin, Wq, Wk, Wv, Wo):
    from concourse import bass_utils

    nc = build_program()
    in_maps = make_in_maps(x, cos, sin, Wq, Wk, Wv, Wo)
    trace = bool(int(os.environ.get("BASS_KERNEL_TRACE", "0")))
    res = bass_utils.run_bass_kernel_spmd(
        nc,
        in_maps,
        core_ids=list(range(8)),
        trace=trace,
    )
    if trace:
        print(f"HW exec time: {res.exec_time_ns} ns")
        if res.instructions_and_trace is not None:
            print(f"trace: {res.instructions_and_trace[1]}")
    out = np.empty((B, S, HID), dtype=np.float32)
    for b in range(B):
        acc = res.results[4 * b]["out"].astype(np.float32)
        for g in range(1, G):
            acc = acc + res.results[4 * b + g]["out"].astype(np.float32)
        out[b] = acc
    return out
